# revision 1
# baseline (speedup 1.0000x reference)
"""Trainium2 Bass kernel for nn_DelayExpansionLayer (histogram_binning).

Computation: per-channel mean of layer_output [64,256,56,56] over (B,H,W),
round to 1e-6, nearest-key lookup in a sorted 1024-entry table, max over
channels, scale by (in_ch*out_ch)/512, broadcast to (56,56).

Strategy (data-parallel over batch, 8 NeuronCores):
  - Each core gets 8 batches = [8,256,56,56] (25.7 MB) and computes
    per-channel partial sums [256] on-device (DMA-bound reduction).
  - Host combines the 8 partial-sum vectors (the tiny [C] all-reduce),
    then does the O(C+K) lookup/max/broadcast epilogue.

Per-core device kernel:
  input  x [8, 128, 2, 3136] f32  (batch, partition, chan-pair, spatial)
         -> each batch slice is one fully-contiguous 3.2 MB DMA
  8x DVE reduce_sum along free dims -> stats[128, 2, 8]
  final reduce over batches -> out [128, 2]  (channel c = 2*p + j)
"""

import sys
import types

import numpy as np

N_CORES = 8
B_FULL, C, H, W = 64, 256, 56, 56
HW = H * W
B_LOCAL = B_FULL // N_CORES
SCALE_DENOM = 32 * 16

# Set by a test harness to enable NTFF tracing of the SPMD run.
TRACE = False
TRACE_TMPDIR = None
LAST_RESULTS = None

_CACHE = {}


def _ensure_axon_hooks_shim():
    """bass_utils' axon trace path imports antenv.axon_hooks; provide a
    no-op shim when the environment's antenv package lacks it."""
    try:
        import antenv.axon_hooks  # noqa: F401
        return
    except ImportError:
        pass
    import antenv

    mod = types.ModuleType("antenv.axon_hooks")
    _hook = [None]
    mod.set_axon_ntff_profile_hook = lambda h: _hook.__setitem__(0, h)
    mod.get_axon_ntff_profile_hook = lambda: _hook[0]
    sys.modules["antenv.axon_hooks"] = mod
    antenv.axon_hooks = mod


def _build():
    if "nc" in _CACHE:
        return _CACHE["nc"]
    import concourse.bacc as bacc
    import concourse.tile as tile
    from concourse import mybir

    nc = bacc.Bacc(
        "TRN2",
        target_bir_lowering=False,
        debug=False,
        enable_asserts=True,
        num_devices=N_CORES,
    )
    x = nc.dram_tensor(
        "x", [B_LOCAL, 128, 2, HW], mybir.dt.float32, kind="ExternalInput"
    ).ap()
    out = nc.dram_tensor(
        "out", [128, 2], mybir.dt.float32, kind="ExternalOutput"
    ).ap()

    with tile.TileContext(nc) as tc:
        with (
            tc.tile_pool(name="inp", bufs=4) as inp,
            tc.tile_pool(name="stats", bufs=1) as sp,
        ):
            stats = sp.tile([128, 2, B_LOCAL], mybir.dt.float32)
            for b in range(B_LOCAL):
                t = inp.tile([128, 2, HW], mybir.dt.float32)
                nc.sync.dma_start(t[:], x[b])
                nc.vector.reduce_sum(
                    stats[:, :, b], t[:], axis=mybir.AxisListType.X
                )
            out_sb = sp.tile([128, 2], mybir.dt.float32)
            nc.vector.reduce_sum(out_sb[:], stats[:], axis=mybir.AxisListType.X)
            nc.sync.dma_start(out[:], out_sb[:])

    nc.compile()
    _CACHE["nc"] = nc
    return nc


def kernel(layer_output, delay_keys, delay_values, in_channels, out_channels):
    global LAST_RESULTS
    _ensure_axon_hooks_shim()
    from concourse.bass_utils import run_bass_kernel_spmd

    x = np.ascontiguousarray(np.asarray(layer_output, dtype=np.float32))
    assert x.shape == (B_FULL, C, H, W), x.shape
    # shard over batch; view channels as (partition, pair): c = 2*p + j
    xr = x.reshape(N_CORES, B_LOCAL, 128, 2, HW)
    in_maps = [{"x": xr[k]} for k in range(N_CORES)]

    nc = _build()
    kwargs = {}
    if TRACE:
        kwargs.update(trace=True, tmpdir=TRACE_TMPDIR)
    res = run_bass_kernel_spmd(nc, in_maps, core_ids=list(range(N_CORES)), **kwargs)
    LAST_RESULTS = res

    # tiny [C] all-reduce of the per-core partial sums
    parts = np.stack([res.results[k]["out"] for k in range(N_CORES)])  # [8,128,2]
    sums = parts.sum(axis=0, dtype=np.float32).reshape(C)  # channel c = 2p+j
    means = sums / np.float32(B_FULL * HW)
    means = np.round(means * np.float32(1e6)) / np.float32(1e6)

    keys = np.asarray(delay_keys, dtype=np.float32)
    values = np.asarray(delay_values, dtype=np.float32)
    K = keys.shape[0]
    idx = np.searchsorted(keys, means)
    lo = np.clip(idx - 1, 0, K - 1)
    hi = np.clip(idx, 0, K - 1)
    pick_hi = np.abs(keys[hi] - means) < np.abs(keys[lo] - means)
    nearest = np.where(pick_hi, hi, lo)
    merged = np.float32(values[nearest].max())

    scale = np.float32(
        (int(np.asarray(in_channels)) * int(np.asarray(out_channels))) / SCALE_DENOM
    )
    return np.full((H, W), merged, dtype=np.float32) * scale
